# revision 5
# baseline (speedup 1.0000x reference)
"""Quantized linear (dynamic per-tensor int8) on 8 TRN2 NeuronCores.

Reference semantics:
    x_q = round(x / s_x), s_x = max|x|/127   (per-tensor, round-half-even)
    w_q = round(w / s_w), s_w = max|w|/127
    out = (x_q @ w_q.T) * (s_x * s_w) + bias

Distribution: data-parallel over M (8 shards of 1024 rows), weight
replicated.  Each core scans a disjoint 1/8 of w and its own x shard for
the local absmax; ONE fused 2-element AllReduce(max) produces both
global scales (a second collective would queue a full ~30us firmware
round behind the first).  Quantized values are exact small integers held
in fp16 (ints <= 2047 are exact in fp16), so the TensorE fp16 matmul
with fp32 PSUM accumulation reproduces the int8 GEMM exactly.

Rounding uses an fp16 magic: (v*inv_s + 1536) written to fp16 rounds the
fractional part half-to-even (ulp = 1 in [1024, 2048)), then an fp16
subtract of 1536 recovers the integer, matching jnp.round.

Scheduling notes:
  * The Tile scheduler is free to reorder DMAs; v2 showed it hoisting
    w-prefetch DMAs above the tail of the x absmax scan, starving the
    scan (which gates the collective trigger) of HBM bandwidth.  Every
    prefetch DMA is therefore "token-gated": a 1-element DVE copy that
    reads lmaxx (the scan's final reduce) writes into the prefetch tile
    first, making the DMA data-dependent on scan completion.
  * All large f32 staging shares ONE ring pool ("stg") so ring WAR also
    sequences the prefetches among themselves in emission order.
  * Strip matmuls are emitted as (mh0, mh1) pairs sharing one stationary
    wq slice; the second of each pair is marked ldweights=True after the
    TileContext exits (the scheduler clones instructions, resetting the
    field).  Marked matmuls lower to a decoupled LDWEIGHTS that the PE's
    reorder window hides, measured at the 216 ns/MM issue roofline.
  * xq is quantized in (mh0, mh1)-interleaved k-chunks, balanced across
    ACT/DVE, racing ~3.4us/chunk-pair production against ~3.8us/pair
    stream consumption, so the stream starts ~2us after the scales land.
"""

import numpy as np

from concourse import bacc, bass_isa
import concourse.bass_utils as bass_utils
import concourse.mybir as mybir
import concourse.tile as tile

P = 128
M, K, N = 8192, 4096, 4096
NCORES = 8
MLOC = M // NCORES  # 1024 rows of x per core
WS = N // NCORES  # 512 columns of wT scanned per core for absmax
MAGIC = 1536.0  # fp16 round-to-int magic: [1024,2048) has ulp 1
MFREE = 512  # moving free dim per matmul (one fp32 PSUM bank)
NSTRIP = 128  # n-columns of w quantized per strip
INV127 = float(np.float32(1.0 / 127.0))

F32 = mybir.dt.float32
F16 = mybir.dt.float16
AX = mybir.AxisListType
ALU = mybir.AluOpType
ACTF = mybir.ActivationFunctionType


def build_body(tc, xT, wT, wscanT, bias, outT, *, n_cores):
    nc = tc.nc
    k, m_loc = xT.shape
    n = wT.shape[1]
    kt_n = k // P  # 32
    n_strips = n // NSTRIP  # 32
    n_ck = kt_n // 4  # 8 quantize chunks of 4 k-tiles per mh half

    paired_mm_names = []

    with (
        tc.tile_pool(name="const", bufs=1) as const,
        tc.tile_pool(name="stats", bufs=1) as stats,
        tc.tile_pool(name="stage", bufs=11) as stage,
        tc.tile_pool(name="xq", bufs=1) as xq_pool,
        tc.tile_pool(name="wq", bufs=5) as wq_pool,
        tc.tile_pool(name="ob", bufs=4) as ob_pool,
        tc.tile_pool(name="ps", bufs=6, space="PSUM") as ps_pool,
        tc.tile_pool(name="dram", bufs=1, space="DRAM") as dram,
    ):
        # ---- bias, laid out bias[s*128+p] -> bias_sb[p, s] ---------------
        bias_sb = const.tile([P, n // P], F32)
        nc.sync.dma_start(bias_sb[:], bias.rearrange("(nt p) -> p nt", p=P))

        xT3 = xT.rearrange("(c p) m -> p c m", p=P)  # [128, 32, 1024]
        wsT3 = wscanT.rearrange("(c p) m -> p c m", p=P)  # [128, 32, 512]
        wT3 = wT.rearrange("(kt p) n -> p kt n", p=P)  # [128, 32, 4096]

        # ---- 1. absmax scans: w (8 MiB) then x (16 MiB), 1 MiB chunks ---
        wmax_cols = stats.tile([P, 8], F32)
        for i in range(8):
            tw = stage.tile([P, 4, WS], F32, tag="stg", name=f"wsc{i}")
            nc.sync.dma_start(tw[:], wsT3[:, i * 4 : (i + 1) * 4, :])
            nc.vector.tensor_reduce(
                wmax_cols[:, i : i + 1], tw[:], axis=AX.XY, op=ALU.max,
                apply_absolute_value=True,
            )
        lmaxw = stats.tile([P, 1], F32)
        nc.vector.tensor_reduce(lmaxw[:], wmax_cols[:], axis=AX.X, op=ALU.max)

        xmax_cols = stats.tile([P, 16], F32)
        for i in range(16):
            tx = stage.tile([P, 2, m_loc], F32, tag="stg", name=f"xsc{i}")
            nc.sync.dma_start(tx[:], xT3[:, i * 2 : (i + 1) * 2, :])
            nc.vector.tensor_reduce(
                xmax_cols[:, i : i + 1], tx[:], axis=AX.XY, op=ALU.max,
                apply_absolute_value=True,
            )
        lmaxx = stats.tile([P, 1], F32)
        nc.vector.tensor_reduce(lmaxx[:], xmax_cols[:], axis=AX.X, op=ALU.max)

        # ---- 2. ONE fused AllReduce(max) over [s_w_raw, s_x_raw] --------
        gmax2 = stats.tile([P, 2], F32)
        nc.gpsimd.partition_all_reduce(
            gmax2[:, 0:1], lmaxw[:], channels=P, reduce_op=bass_isa.ReduceOp.max,
        )
        nc.gpsimd.partition_all_reduce(
            gmax2[:, 1:2], lmaxx[:], channels=P, reduce_op=bass_isa.ReduceOp.max,
        )
        cc_in = dram.tile([1, 2], F32)
        cc_out = dram.tile([1, 2], F32)
        # Tiny collective DMAs ride the otherwise-idle Activation HWDGE ring
        # so they don't queue behind in-flight multi-MiB staging packets.
        nc.scalar.dma_start(cc_in[:], gmax2[0:1, :])
        nc.gpsimd.collective_compute(
            "AllReduce", ALU.max, replica_groups=[list(range(n_cores))],
            ins=[cc_in.opt()], outs=[cc_out.opt()],
        )

        # ---- 3. token-gated prefetch through the staging ring -----------
        # The token write makes each prefetch DMA data-dependent on the
        # scan's final reduce so the scheduler cannot hoist it into the
        # scan's HBM bandwidth window.
        wf_tiles = {}  # (s, half) -> f32 stage tile [P, 16, 128]
        xre_tiles = {}  # (mh, ck) -> f32 stage tile [P, 4, 512]

        def load_wf(s):
            for h in range(2):
                t = stage.tile([P, 16, NSTRIP], F32, tag="stg", name=f"wf{s}_{h}")
                nc.vector.tensor_copy(t[0:1, 0:1, 0:1], lmaxx[0:1, 0:1])
                nc.sync.dma_start(
                    t[:],
                    wT3[:, h * 16 : (h + 1) * 16,
                        s * NSTRIP : (s + 1) * NSTRIP],
                )
                wf_tiles[(s, h)] = t

        def load_xre(mh, ck):
            t = stage.tile([P, 4, MFREE], F32, tag="stg", name=f"xr{mh}_{ck}")
            nc.vector.tensor_copy(t[0:1, 0:1, 0:1], lmaxx[0:1, 0:1])
            nc.sync.dma_start(
                t[:],
                xT3[:, ck * 4 : (ck + 1) * 4,
                    mh * MFREE : (mh + 1) * MFREE],
            )
            xre_tiles[(mh, ck)] = t

        load_wf(0)
        load_wf(1)
        for ck in range(n_ck):
            load_xre(0, ck)
            load_xre(1, ck)
        load_wf(2)
        load_wf(3)

        # ---- 4. scales from the collective result -----------------------
        gsb2 = stats.tile([1, 2], F32)
        nc.scalar.dma_start(gsb2[:], cc_out[:])
        rec2 = stats.tile([1, 2], F32)
        sc3 = stats.tile([1, 3], F32)  # [inv_sw, inv_sx, out_sc-partial]
        nc.vector.reciprocal(rec2[:], gsb2[:])
        nc.vector.tensor_scalar(sc3[:, 0:2], rec2[:], 127.0, None, op0=ALU.mult)
        # out_sc = (gw/127) * (gx/127)
        prod = stats.tile([1, 1], F32)
        nc.vector.tensor_tensor(prod[:], gsb2[:, 0:1], gsb2[:, 1:2], op=ALU.mult)
        nc.vector.tensor_scalar(
            sc3[:, 2:3], prod[:], INV127 * INV127, None, op0=ALU.mult
        )
        scb = const.tile([P, 3], F32)
        nc.gpsimd.partition_broadcast(scb[:], sc3[:])
        inv_sw = scb[:, 0:1]
        inv_sx = scb[:, 1:2]
        out_sc = scb[:, 2:3]

        # ---- 5. quantize: wq strips 0-3 interleaved with xq chunk pairs -
        wq_tiles = {}

        def quant_w_strip(s, on_act=True):
            wq = wq_pool.tile([P, kt_n, NSTRIP], F16, tag="wq", name=f"wq{s}")
            for h in range(2):
                sl = wq[:, h * 16 : (h + 1) * 16, :]
                src = wf_tiles.pop((s, h))[:]
                if on_act:
                    nc.scalar.activation(
                        sl, src, ACTF.Copy, bias=MAGIC, scale=inv_sw
                    )
                else:
                    nc.vector.tensor_scalar(
                        sl, src, inv_sw, MAGIC, op0=ALU.mult, op1=ALU.add
                    )
                nc.vector.tensor_scalar(sl, sl, MAGIC, None, op0=ALU.subtract)
            wq_tiles[s] = wq

        xqs = [
            xq_pool.tile([P, kt_n, MFREE], F16, tag=f"xq{h}", name=f"xq{h}")
            for h in range(2)
        ]

        def quant_x_pair(ck):
            # mh0 pass1 on DVE, its subtract on ACT; mh1 pass1 on ACT,
            # its subtract on DVE: ~3.3-3.4us per pair on each engine.
            sl0 = xqs[0][:, ck * 4 : (ck + 1) * 4, :]
            src0 = xre_tiles.pop((0, ck))[:]
            nc.vector.tensor_scalar(
                sl0, src0, inv_sx, MAGIC, op0=ALU.mult, op1=ALU.add
            )
            sl1 = xqs[1][:, ck * 4 : (ck + 1) * 4, :]
            src1 = xre_tiles.pop((1, ck))[:]
            nc.scalar.activation(sl1, src1, ACTF.Copy, bias=MAGIC, scale=inv_sx)
            nc.scalar.activation(sl0, sl0, ACTF.Copy, bias=-MAGIC)
            nc.vector.tensor_scalar(sl1, sl1, MAGIC, None, op0=ALU.subtract)

        quant_w_strip(0, on_act=False)
        quant_x_pair(0)
        quant_w_strip(1)
        quant_x_pair(1)
        quant_w_strip(2, on_act=False)
        quant_x_pair(2)
        quant_w_strip(3)
        for ck in range(3, n_ck):
            quant_x_pair(ck)

        # ---- 6. paired stream over strips ------------------------------
        # The wq quantize for strip s+4 is emitted BEFORE the matmuls of
        # strip s so the DVE/ACT FIFOs never make the TensorE wait.
        for s in range(n_strips):
            if s + 4 < n_strips:
                load_wf(s + 4)
                quant_w_strip(s + 4, on_act=(s % 2 == 0))
            wq = wq_tiles[s]
            ps0 = ps_pool.tile([P, MFREE], F32, tag="ps", name=f"ps{s}_0")
            ps1 = ps_pool.tile([P, MFREE], F32, tag="ps", name=f"ps{s}_1")
            for kt in range(kt_n):
                i1 = nc.tensor.matmul(
                    ps0[:], wq[:, kt, :], xqs[0][:, kt, :],
                    start=(kt == 0), stop=(kt == kt_n - 1),
                )
                i2 = nc.tensor.matmul(
                    ps1[:], wq[:, kt, :], xqs[1][:, kt, :],
                    start=(kt == 0), stop=(kt == kt_n - 1),
                )
                paired_mm_names.append(i1.ins.name)
                paired_mm_names.append(i2.ins.name)
            del wq_tiles[s]
            ob0 = ob_pool.tile([P, MFREE], F32, tag="ob")
            nc.vector.tensor_scalar(
                ob0[:], ps0[:], out_sc, bias_sb[:, s : s + 1],
                op0=ALU.mult, op1=ALU.add,
            )
            nc.gpsimd.dma_start(
                outT[s * NSTRIP : (s + 1) * NSTRIP, 0:MFREE], ob0[:],
            )
            ob1 = ob_pool.tile([P, MFREE], F32, tag="ob")
            nc.vector.tensor_scalar(
                ob1[:], ps1[:], out_sc, bias_sb[:, s : s + 1],
                op0=ALU.mult, op1=ALU.add,
            )
            nc.gpsimd.dma_start(
                outT[s * NSTRIP : (s + 1) * NSTRIP, MFREE : 2 * MFREE], ob1[:],
            )

    return paired_mm_names


def build_nc(m_loc=MLOC, k=K, n=N, ws=WS, n_cores=NCORES):
    nc = bacc.Bacc("TRN2", target_bir_lowering=False, debug=False,
                   num_devices=n_cores)
    xT = nc.dram_tensor("xT", [k, m_loc], F32, kind="ExternalInput").ap()
    wT = nc.dram_tensor("wT", [k, n], F32, kind="ExternalInput").ap()
    wscanT = nc.dram_tensor("wscanT", [k, ws], F32, kind="ExternalInput").ap()
    bias = nc.dram_tensor("bias", [n], F32, kind="ExternalInput").ap()
    outT = nc.dram_tensor("outT", [n, m_loc], F32, kind="ExternalOutput").ap()
    with tile.TileContext(nc) as tc:
        paired = build_body(tc, xT, wT, wscanT, bias, outT, n_cores=n_cores)
    # Mark the second matmul of each weight-sharing pair ldweights=True
    # (must happen after TileContext exit: the scheduler clones
    # instructions and resets the field).
    mark = set(paired)
    for fn in nc.m.functions:
        for bb in fn.blocks:
            for inst in bb.instructions:
                if inst.name in mark:
                    inst.ldweights = True
    nc.compile()
    return nc


def make_in_maps(x, weight, bias, n_cores=NCORES):
    m_loc = x.shape[0] // n_cores
    ws = weight.shape[0] // n_cores
    wT = np.ascontiguousarray(weight.T)
    bias = np.ascontiguousarray(bias, dtype=np.float32)
    maps = []
    for c in range(n_cores):
        maps.append({
            "xT": np.ascontiguousarray(x[c * m_loc : (c + 1) * m_loc].T),
            "wT": wT,
            "wscanT": np.ascontiguousarray(weight[c * ws : (c + 1) * ws].T),
            "bias": bias,
        })
    return maps


_NC_CACHE = {}
LAST_RUN = None


def kernel(x, weight, bias, _trace=False):
    global LAST_RUN
    x = np.ascontiguousarray(np.asarray(x), dtype=np.float32)
    weight = np.ascontiguousarray(np.asarray(weight), dtype=np.float32)
    bias = np.asarray(bias, dtype=np.float32)
    if "full" not in _NC_CACHE:
        _NC_CACHE["full"] = build_nc()
    nc = _NC_CACHE["full"]
    in_maps = make_in_maps(x, weight, bias)
    res = bass_utils.run_bass_kernel_spmd(
        nc, in_maps, core_ids=list(range(NCORES)), trace=_trace
    )
    LAST_RUN = res
    out = np.empty((M, N), np.float32)
    for c in range(NCORES):
        out[c * MLOC : (c + 1) * MLOC, :] = res.results[c]["outT"].T
    return out
